# revision 8
# baseline (speedup 1.0000x reference)
"""Trainium2 Bass kernel for nn_LongRangeInteraction (segment reduce over structures).

Sharding: data-parallel over structures. 32 structures / 8 cores = 4 per core.
Atoms of each structure are padded to P (multiple of 128); padded atoms have
pos=0, h=0 so they contribute nothing to the segment sums, and their output
rows are dropped on the host.

Per-core math (all on device):
  u     = (pos/2pi) @ kv^T                      [A, K]   (PE, packed per atom-block)
  gs    = u mod 1 ; gc = (u + 0.25) mod 1       (DVE)
  S'    = sin(2pi*gs - pi) = -sin(theta)        (ACT Sin)
  C'    = sin(2pi*gc - pi) = -cos(theta)        (ACT Sin)
  F^T   = MLP(kv) with tanh-based exact gelu    (PE + ACT Tanh/Identity + DVE helpers)
  Sre'  = C'^T @ H ; Ssum' = S'^T @ H           [K, D]  (PE accumulate)
  U     = F*Sre' ; V = F*Ssum' ; Vn = -V        (DVE)
  reT   = U^T @ C'T + V^T @ S'T                 [D, A]  (PE, N=P)
  imT   = U^T @ S'T + Vn^T @ C'T                [D, A]
Global sign of (C', S') = (-cos, -sin) cancels because every output term is
bilinear in the trig factors.  Output assembled host-side as re + 1j*im.
"""

import math

import numpy as np

import concourse.bacc as bacc
import concourse.bass as bass
import concourse.mybir as mybir
import concourse.tile as tile
from concourse.bass_utils import run_bass_kernel_spmd

F32 = mybir.dt.float32
I32 = mybir.dt.int32
AF = mybir.ActivationFunctionType
ALU = mybir.AluOpType

N_ATOMS, B, K, D = 8192, 32, 128, 128
NCORES, NS = 8, 4  # cores, structures per core
TWO_PI = 2.0 * math.pi

# tanh-gelu: gelu(x) = 0.5*x*(1 + tanh(c_mul*(x^2 + c_add)*x))
GELU_C_MUL = 0.7978845608028654 * 0.044715
GELU_C_ADD = 1.0 / 0.044715


def build_kernel(P: int):
    """Build the per-core Bass program for structure padding P (multiple of 128)."""
    nb = P // 128
    AT = NS * P  # padded atoms per core

    nc = bacc.Bacc("TRN2", target_bir_lowering=False, debug=False,
                   num_devices=NCORES)

    # ---- DRAM I/O ----
    posTs_d = nc.dram_tensor("posTs", [3, AT], F32, kind="ExternalInput")
    h_d = nc.dram_tensor("h", [AT, D], F32, kind="ExternalInput")
    kvT_d = nc.dram_tensor("kvT", [3, NS * K], F32, kind="ExternalInput")
    w1_d = nc.dram_tensor("w1", [3, D], F32, kind="ExternalInput")
    w2_d = nc.dram_tensor("w2", [D, D], F32, kind="ExternalInput")
    w3_d = nc.dram_tensor("w3", [D, D], F32, kind="ExternalInput")
    b1_d = nc.dram_tensor("b1", [D, 1], F32, kind="ExternalInput")
    b2_d = nc.dram_tensor("b2", [D, 1], F32, kind="ExternalInput")
    b3_d = nc.dram_tensor("b3", [D, 1], F32, kind="ExternalInput")
    ident_d = nc.dram_tensor("ident", [128, 128], F32, kind="ExternalInput")
    outRe_d = nc.dram_tensor("outRe", [D, AT], F32, kind="ExternalOutput")
    outIm_d = nc.dram_tensor("outIm", [D, AT], F32, kind="ExternalOutput")

    with tile.TileContext(nc) as tc:
        with (
            tc.tile_pool(name="const", bufs=1) as cpool,
            tc.tile_pool(name="work", bufs=1) as wpool,
            tc.tile_pool(name="ps_big", bufs=2, space=bass.MemorySpace.PSUM) as ps_big,
            tc.tile_pool(name="ps_p1", bufs=2, space=bass.MemorySpace.PSUM) as ps_p1,
            tc.tile_pool(name="ps_out", bufs=2, space=bass.MemorySpace.PSUM) as ps_out,
        ):
            # ---- load constants ----
            def cload(dram, shape, tag):
                t = cpool.tile(shape, F32, tag=tag)
                nc.sync.dma_start(t[:], dram[:])
                return t

            posTs = cload(posTs_d, [3, AT], "posTs")
            kvT = cload(kvT_d, [3, NS * K], "kvT")
            w1 = cload(w1_d, [3, D], "w1")
            w2 = cload(w2_d, [D, D], "w2")
            w3 = cload(w3_d, [D, D], "w3")
            b1 = cload(b1_d, [D, 1], "b1")
            b2 = cload(b2_d, [D, 1], "b2")
            b3 = cload(b3_d, [D, 1], "b3")
            ident = cload(ident_d, [128, 128], "ident")
            h_sb = cpool.tile([128, nb * NS * 128], F32, tag="h")
            # block j of h (rows j*128..) -> cols j*128..
            for j in range(NS * nb):
                nc.sync.dma_start(h_sb[:, j * 128:(j + 1) * 128],
                                  h_d[j * 128:(j + 1) * 128, :])

            # ---- MLP: FT[d, s*K+k] (transposed activations; lhsT = W) ----
            def gelu(tag, z_ps, bias):
                """x = Identity(z+bias); return 2*gelu(x) in sbuf [128, NS*K]."""
                x = wpool.tile([128, NS * K], F32, tag=f"{tag}_x")
                nc.scalar.activation(x[:], z_ps[:], AF.Identity, bias=bias[:])
                x2 = wpool.tile([128, NS * K], F32, tag=f"{tag}_x2")
                nc.vector.tensor_mul(x2[:], x[:], x[:])
                g = wpool.tile([128, NS * K], F32, tag=f"{tag}_g")
                nc.vector.scalar_tensor_tensor(g[:], x2[:], GELU_C_ADD, x[:],
                                               op0=ALU.add, op1=ALU.mult)
                t = wpool.tile([128, NS * K], F32, tag=f"{tag}_t")
                nc.scalar.activation(t[:], g[:], AF.Tanh, scale=GELU_C_MUL)
                q = wpool.tile([128, NS * K], F32, tag=f"{tag}_q")
                nc.vector.scalar_tensor_tensor(q[:], t[:], 1.0, x[:],
                                               op0=ALU.add, op1=ALU.mult)
                return q

            z1 = ps_big.tile([128, NS * K], F32, tag="mm512")
            nc.tensor.matmul(z1[:], w1[:], kvT[:], start=True, stop=True)
            a1 = gelu("l1", z1, b1)  # = 2*gelu -> w2 host-scaled by 1/2
            z2 = ps_big.tile([128, NS * K], F32, tag="mm512")
            nc.tensor.matmul(z2[:], w2[:], a1[:], start=True, stop=True)
            a2 = gelu("l2", z2, b2)  # = 2*gelu -> w3 host-scaled by 1/2
            z3 = ps_big.tile([128, NS * K], F32, tag="mm512")
            nc.tensor.matmul(z3[:], w3[:], a2[:], start=True, stop=True)
            FT = wpool.tile([128, NS * K], F32, tag="FT")
            nc.scalar.activation(FT[:], z3[:], AF.Identity, bias=b3[:])

            # F[s] = FT[:, s*K:(s+1)*K]^T  via PE transpose
            F_sb = []
            for s in range(NS):
                fp = ps_out.tile([128, 128], F32, tag="out_ps")
                nc.tensor.transpose(fp[:], FT[:, s * K:(s + 1) * K], ident[:])
                f = wpool.tile([128, 128], F32, tag=f"F{s}")
                nc.vector.tensor_copy(f[:], fp[:])
                F_sb.append(f)

            # ---- theta -> reduced args -> trig, packed by atom-block index ----
            C_sb, S_sb = [], []  # [nb] tiles [128, NS*K]; col group s = structure s
            for bi in range(nb):
                u = ps_big.tile([128, NS * K], F32, tag="mm512")
                for s in range(NS):
                    nc.tensor.matmul(
                        u[:, s * K:(s + 1) * K],
                        posTs[:, s * P + bi * 128: s * P + (bi + 1) * 128],
                        kvT[:, s * K:(s + 1) * K],
                        start=True, stop=True)
                # range reduction: r = u - round(u) in [-0.5, 0.5]
                ms = wpool.tile([128, NS * K], I32, tag=f"ms{bi}")
                nc.vector.tensor_copy(ms[:], u[:])
                gs = wpool.tile([128, NS * K], F32, tag=f"gs{bi}")
                nc.vector.tensor_sub(gs[:], u[:], ms[:])
                mc = wpool.tile([128, NS * K], I32, tag=f"mc{bi}")
                nc.vector.tensor_scalar(mc[:], u[:], 0.25, None, op0=ALU.add)
                gc = wpool.tile([128, NS * K], F32, tag=f"gc{bi}")
                nc.vector.scalar_tensor_tensor(gc[:], u[:], 0.25, mc[:],
                                               op0=ALU.add, op1=ALU.subtract)
                Sb = wpool.tile([128, NS * K], F32, tag=f"S{bi}")
                nc.scalar.activation(Sb[:], gs[:], AF.Sin, scale=TWO_PI)
                Cb = wpool.tile([128, NS * K], F32, tag=f"C{bi}")
                nc.scalar.activation(Cb[:], gc[:], AF.Sin, scale=TWO_PI)
                C_sb.append(Cb)
                S_sb.append(Sb)

            # ---- per structure: phase1, U/V, transposes, phase3, out ----
            for s in range(NS):
                sre = ps_p1.tile([128, 128], F32, tag="p1")
                ssum = ps_p1.tile([128, 128], F32, tag="p1")
                for bi in range(nb):
                    hblk = h_sb[:, (s * nb + bi) * 128:(s * nb + bi + 1) * 128]
                    nc.tensor.matmul(sre[:], C_sb[bi][:, s * K:(s + 1) * K],
                                     hblk, start=(bi == 0), stop=(bi == nb - 1))
                for bi in range(nb):
                    hblk = h_sb[:, (s * nb + bi) * 128:(s * nb + bi + 1) * 128]
                    nc.tensor.matmul(ssum[:], S_sb[bi][:, s * K:(s + 1) * K],
                                     hblk, start=(bi == 0), stop=(bi == nb - 1))

                U = wpool.tile([128, 128], F32, tag=f"U{s}")
                nc.vector.tensor_mul(U[:], F_sb[s][:], sre[:])
                V = wpool.tile([128, 128], F32, tag=f"V{s}")
                nc.vector.tensor_mul(V[:], F_sb[s][:], ssum[:])
                Vn = wpool.tile([128, 128], F32, tag=f"Vn{s}")
                nc.vector.scalar_tensor_tensor(Vn[:], ssum[:], -1.0, F_sb[s][:],
                                               op0=ALU.mult, op1=ALU.mult)

                # transpose C/S blocks of this structure -> [K, P] in sbuf
                ct_ps = ps_big.tile([128, 512], F32, tag="t_ps")
                st_ps = ps_big.tile([128, 512], F32, tag="t_ps")
                for bi in range(nb):
                    nc.tensor.transpose(ct_ps[:, bi * 128:(bi + 1) * 128],
                                        C_sb[bi][:, s * K:(s + 1) * K], ident[:])
                    nc.tensor.transpose(st_ps[:, bi * 128:(bi + 1) * 128],
                                        S_sb[bi][:, s * K:(s + 1) * K], ident[:])
                CT = wpool.tile([128, P], F32, tag=f"CT{s}")
                nc.vector.tensor_copy(CT[:], ct_ps[:, :P])
                ST = wpool.tile([128, P], F32, tag=f"ST{s}")
                nc.vector.tensor_copy(ST[:], st_ps[:, :P])

                reT = ps_out.tile([128, P], F32, tag="out_ps")
                nc.tensor.matmul(reT[:], U[:], CT[:], start=True, stop=False)
                nc.tensor.matmul(reT[:], V[:], ST[:], start=False, stop=True)
                imT = ps_out.tile([128, P], F32, tag="out_ps")
                nc.tensor.matmul(imT[:], U[:], ST[:], start=True, stop=False)
                nc.tensor.matmul(imT[:], Vn[:], CT[:], start=False, stop=True)

                re_sb = wpool.tile([128, P], F32, tag=f"re{s}")
                nc.scalar.activation(re_sb[:], reT[:], AF.Copy)
                im_sb = wpool.tile([128, P], F32, tag=f"im{s}")
                nc.scalar.activation(im_sb[:], imT[:], AF.Copy)
                nc.sync.dma_start(outRe_d[:, s * P:(s + 1) * P], re_sb[:])
                nc.sync.dma_start(outIm_d[:, s * P:(s + 1) * P], im_sb[:])

    nc.compile()
    return nc


def shard_inputs(inputs):
    """Host-side sharding: pad each structure to P atoms, build per-core maps."""
    batch = np.asarray(inputs["batch"])
    counts = np.bincount(batch, minlength=B).astype(np.int64)
    starts = np.concatenate([[0], np.cumsum(counts)])
    P = int(max(1, int(np.ceil(counts.max() / 128)))) * 128
    AT = NS * P

    pos = np.ascontiguousarray(np.asarray(inputs["positions"], np.float32))
    h = np.ascontiguousarray(np.asarray(inputs["h"], np.float32))
    kv = np.asarray(inputs["k_vectors"], np.float32)
    w1 = np.ascontiguousarray(np.asarray(inputs["W1"], np.float32))
    w2 = np.ascontiguousarray(np.asarray(inputs["W2"], np.float32) * 0.5)
    w3 = np.ascontiguousarray(np.asarray(inputs["W3"], np.float32) * 0.5)
    b1 = np.ascontiguousarray(np.asarray(inputs["b1"], np.float32)[:, None])
    b2 = np.ascontiguousarray(np.asarray(inputs["b2"], np.float32)[:, None])
    b3 = np.ascontiguousarray(np.asarray(inputs["b3"], np.float32)[:, None])
    ident = np.eye(128, dtype=np.float32)

    in_maps = []
    for c in range(NCORES):
        posp = np.zeros((AT, 3), np.float32)
        hp = np.zeros((AT, D), np.float32)
        for j in range(NS):
            bidx = NS * c + j
            n = int(counts[bidx])
            posp[j * P:j * P + n] = pos[starts[bidx]:starts[bidx] + n]
            hp[j * P:j * P + n] = h[starts[bidx]:starts[bidx] + n]
        kvT = np.ascontiguousarray(
            kv[NS * c:NS * c + NS].transpose(2, 0, 1).reshape(3, NS * K))
        in_maps.append({
            "posTs": np.ascontiguousarray(posp.T / TWO_PI),
            "h": hp,
            "kvT": kvT,
            "w1": w1, "w2": w2, "w3": w3,
            "b1": b1, "b2": b2, "b3": b3,
            "ident": ident,
        })
    return in_maps, counts, starts, P


_CACHE = {}


def kernel(k_vectors, positions, h, W1, b1, W2, b2, W3, b3, batch,
           _trace=False, _tmpdir=None):
    inputs = dict(k_vectors=k_vectors, positions=positions, h=h, W1=W1, b1=b1,
                  W2=W2, b2=b2, W3=W3, b3=b3, batch=batch)
    in_maps, counts, starts, P = shard_inputs(inputs)
    if P not in _CACHE:
        _CACHE[P] = build_kernel(P)
    nc = _CACHE[P]
    res = run_bass_kernel_spmd(nc, in_maps, core_ids=list(range(NCORES)),
                               trace=_trace, tmpdir=_tmpdir)
    out = np.zeros((N_ATOMS, D), np.complex64)
    for c in range(NCORES):
        reT = res.results[c]["outRe"]
        imT = res.results[c]["outIm"]
        for j in range(NS):
            bidx = NS * c + j
            n = int(counts[bidx])
            out[starts[bidx]:starts[bidx] + n] = (
                reT[:, j * P:j * P + n] + 1j * imT[:, j * P:j * P + n]).T
    if _trace:
        kernel.last_results = res
    return out
